# revision 28
# baseline (speedup 1.0000x reference)
"""Trainium2 Bass kernel for paged-attention Llama-style block (nn_L4maAttention).

Sharding: tensor-parallel over heads across 8 NeuronCores. Core c owns
q-heads [4c, 4c+4), kv-head c, wq/wk/wv row shards and the matching wo
column shard. Each core computes a full [T, HID] partial of the output
projection (bf16); the host sums the 8 partials (the TP reduce).

v2: fused per-chunk pipeline in bf16. For each 512-token chunk:
  P1: QKV projections (bf16 matmuls, fp32 PSUM) + Llama-3.1 RoPE on Q/K
      (fused halfswap via partition-shifted DVE muls) + V transpose via
      DRAM-roundtrip XBAR DMA. PSUM accumulators rotate through a
      3-bank pool so the PE never stalls on drains.
  A:  causal attention with transposed scores [k on partitions]; exp on
      ACT (bf16 out); causal mask-mul on DVE (bf16 2x); denominator
      accumulated on GpSimd (fp32), reduced via a ones-matmul into the
      recycled score-PSUM pool; reciprocal_approx_fast on DVE.
  P3: output projection partial; PSUM drained to bf16 alternating
      ACT/DVE; P3 of the previous chunk is interleaved between attention
      heads to fill PE bubbles.
The fused stream keeps the PE continuously busy so the HAM activity
throttle stays at full speed (the v1 kernel lost ~35% of its runtime to
k=4/8 half-throttle windows at phase boundaries).
"""

import math
import sys
from contextlib import ExitStack

import numpy as np

for _p in ("/opt/trn_rl_repo",):
    if _p not in sys.path:
        sys.path.insert(0, _p)

import ml_dtypes  # noqa: E402

import concourse.mybir as mybir  # noqa: E402
import concourse.tile as tile  # noqa: E402
from concourse import bacc  # noqa: E402
from concourse.bass_utils import run_bass_kernel_spmd  # noqa: E402

NCORES = 8
HID = 4096
NH = 32
NKV = 8
HD = 128
B = 4
S = 1024
T = B * S
GQ = NH // NCORES          # q heads per core = 4
DQ = GQ * HD               # 512
TCH = 512                  # token chunk
NTCH = T // TCH            # 8
KT = HID // 128            # 32 contraction tiles
SCALE = 1.0 / math.sqrt(HD)

FP32 = mybir.dt.float32
FP32R = mybir.dt.float32r
BF16 = mybir.dt.bfloat16
NP_BF16 = ml_dtypes.bfloat16

_PROG_CACHE: dict = {}


def _llama31_freqs_np(head_dim: int) -> np.ndarray:
    half = head_dim // 2
    theta, scale, low_ff, high_ff, old_ctx = 500000.0, 8.0, 1.0, 4.0, 8192.0
    freq = 1.0 / (theta ** (np.arange(half, dtype=np.float64) * 2.0 / head_dim))
    wavelen = 2.0 * np.pi / freq
    low_wl, high_wl = old_ctx / low_ff, old_ctx / high_ff
    smooth = (old_ctx / wavelen - low_ff) / (high_ff - low_ff)
    out = np.where(
        wavelen < high_wl,
        freq,
        np.where(wavelen > low_wl, freq / scale, (1.0 - smooth) * freq / scale + smooth * freq),
    )
    return out.astype(np.float64)


def _rope_tables(pos: np.ndarray) -> tuple[np.ndarray, np.ndarray]:
    """cosF [128, n]: cos duplicated on both partition halves.
    sinF2 [128, n]: -sin on rows 0-63, +sin on rows 64-127. The kernel
    computes out = x*cosF + t2 with t2[0:64] = x[64:128]*sinF2[0:64] and
    t2[64:128] = x[0:64]*sinF2[64:128], which equals rotate-half RoPE."""
    freqs = _llama31_freqs_np(HD)
    ang = pos.astype(np.float64)[None, :] * freqs[:, None]  # [64, n]
    c = np.cos(ang).astype(np.float32)
    s = np.sin(ang).astype(np.float32)
    cosF = np.concatenate([c, c], axis=0)
    sinF2 = np.concatenate([-s, s], axis=0)
    return np.ascontiguousarray(cosF), np.ascontiguousarray(sinF2)


def _masks_np() -> np.ndarray:
    """4 diagonal-block masks [128, 4*512]: mask[r][j, i] = (128*r + j) <= i."""
    m = np.zeros((128, 4 * TCH), np.float32)
    j = np.arange(128)[:, None]
    i = np.arange(TCH)[None, :]
    for r in range(4):
        m[:, r * TCH:(r + 1) * TCH] = ((128 * r + j) <= i).astype(np.float32)
    return m.astype(NP_BF16)


def _tile_h(h: np.ndarray) -> np.ndarray:
    """[T, HID] fp32 -> [NTCH, 128, KT, TCH] bf16 with
    out[tci, p, k, t] = h[tci*TCH + t, k*128 + p]."""
    a = h.reshape(NTCH, TCH, KT, 128).transpose(0, 3, 2, 1)
    return np.ascontiguousarray(a.astype(NP_BF16))


def _tile_w(wT: np.ndarray) -> np.ndarray:
    """[HID, D] fp32 -> [128, KT, D] bf16 with out[p, k, d] = wT[k*128+p, d]."""
    a = wT.reshape(KT, 128, -1).transpose(1, 0, 2)
    return np.ascontiguousarray(a.astype(NP_BF16))


def _tile_wo(woT: np.ndarray) -> np.ndarray:
    """[DQ, HID] fp32 -> [128, GQ, HID] bf16 with out[p, g, e] = woT[g*128+p, e]."""
    a = woT.reshape(GQ, 128, HID).transpose(1, 0, 2)
    return np.ascontiguousarray(a.astype(NP_BF16))


def _build_program(split_kv: bool):
    nc = bacc.Bacc(
        "TRN2",
        target_bir_lowering=False,
        debug=False,
        enable_asserts=False,
        num_devices=NCORES,
    )
    h16 = nc.dram_tensor("h16", [NTCH, 128, KT, TCH], BF16, kind="ExternalInput")
    h16kv = (
        nc.dram_tensor("h16kv", [NTCH, 128, KT, TCH], BF16, kind="ExternalInput")
        if split_kv
        else h16
    )
    wq16 = nc.dram_tensor("wq16", [128, KT, DQ], BF16, kind="ExternalInput")
    wk16 = nc.dram_tensor("wk16", [128, KT, HD], BF16, kind="ExternalInput")
    wv16 = nc.dram_tensor("wv16", [128, KT, HD], BF16, kind="ExternalInput")
    wo16 = nc.dram_tensor("wo16", [128, GQ, HID], BF16, kind="ExternalInput")
    cosq = nc.dram_tensor("cosq", [128, T], FP32, kind="ExternalInput")
    sinq = nc.dram_tensor("sinq", [128, T], FP32, kind="ExternalInput")
    if split_kv:
        cosk = nc.dram_tensor("cosk", [128, T], FP32, kind="ExternalInput")
        sink = nc.dram_tensor("sink", [128, T], FP32, kind="ExternalInput")
    else:
        cosk, sink = cosq, sinq
    maskd = nc.dram_tensor("maskd", [128, 4 * TCH], BF16, kind="ExternalInput")
    onesd = nc.dram_tensor("onesd", [128, 128], FP32R, kind="ExternalInput")
    outp = nc.dram_tensor("outp", [T, HID], BF16, kind="ExternalOutput")

    with tile.TileContext(nc) as tc, ExitStack() as ctx:
        const_pool = ctx.enter_context(tc.tile_pool(name="const", bufs=1))
        dram_pool = ctx.enter_context(tc.tile_pool(name="dram", bufs=2, space="DRAM"))
        res_pool = ctx.enter_context(tc.tile_pool(name="res", bufs=1))

        ones_sb = const_pool.tile([128, 128], FP32R)
        nc.sync.dma_start(ones_sb[:], onesd.ap()[:, :])
        mask_sb = const_pool.tile([128, 4 * TCH], BF16)
        nc.sync.dma_start(mask_sb[:], maskd.ap()[:, :])

        # resident weights; DMAs are emitted inside p1(0) after the first h
        # chunk so the startup critical path (h0 + wk) isn't queued behind wq
        wq_sb = res_pool.tile([128, KT, DQ], BF16)
        wk_sb = res_pool.tile([128, KT, HD], BF16)
        wv_sb = res_pool.tile([128, KT, HD], BF16)



        KT_res = res_pool.tile([128, T], BF16)          # 8KB/part
        V_res = res_pool.tile([128, KT, HD], BF16)      # 8KB/part

        hb = 1 if split_kv else 2
        pools = {
            "h": ctx.enter_context(tc.tile_pool(name="h", bufs=hb)),
            "cs": ctx.enter_context(tc.tile_pool(name="cs", bufs=2)),
            "q": ctx.enter_context(tc.tile_pool(name="q", bufs=2)),
            "t": ctx.enter_context(tc.tile_pool(name="t", bufs=1)),
            "vsb": ctx.enter_context(tc.tile_pool(name="vsb", bufs=2)),
            "ex": ctx.enter_context(tc.tile_pool(name="ex", bufs=5)),
            "es": ctx.enter_context(tc.tile_pool(name="es", bufs=2)),
            "rec": ctx.enter_context(tc.tile_pool(name="rec", bufs=2)),
            "ao": ctx.enter_context(tc.tile_pool(name="ao", bufs=2)),
            "wo": ctx.enter_context(tc.tile_pool(name="wo", bufs=3)),
            "ob": ctx.enter_context(tc.tile_pool(name="ob", bufs=4)),
            "proj": ctx.enter_context(tc.tile_pool(name="proj", bufs=2, space="PSUM")),
            "pss": ctx.enter_context(tc.tile_pool(name="pss", bufs=2, space="PSUM")),
            "pv": ctx.enter_context(tc.tile_pool(name="pv", bufs=1, space="PSUM")),
            "pso": ctx.enter_context(tc.tile_pool(name="pso", bufs=3, space="PSUM")),
        }
        if split_kv:
            pools["hkv"] = ctx.enter_context(tc.tile_pool(name="hkv", bufs=1))

        def rope(ps, cos_t, sin_t, out_ap):
            # out[0:64] = x1*cos - x2*sin, out[64:128] = x2*cos + x1*sin via
            # partition-shifted muls (no DMA in the rope dependency chain)
            t1 = pools["t"].tile([128, TCH], FP32, tag="t1")
            t2 = pools["t"].tile([128, TCH], FP32, tag="t2")
            nc.vector.tensor_mul(t1[:], ps[:], cos_t[:])
            nc.vector.tensor_mul(t2[0:64, :], ps[64:128, :], sin_t[0:64, :])
            nc.vector.tensor_mul(t2[64:128, :], ps[0:64, :], sin_t[64:128, :])
            nc.vector.tensor_add(out_ap, t1[:], t2[:])

        def p1(tci, qT):
            """projections + rope for chunk tci; writes qT[g], KT_res, V_res."""
            tsl = slice(tci * TCH, (tci + 1) * TCH)
            if tci == 0:
                # K/V-projection weights first: chunk 0's first matmuls need
                # wk + the first h quarter, so those lead the DMA queue
                nc.sync.dma_start(wk_sb[:], wk16.ap()[:, :, :])
                nc.sync.dma_start(wv_sb[:], wv16.ap()[:, :, :])
            h_t = pools["h"].tile([128, KT, TCH], BF16, tag="h")
            for kg in range(0, KT, 8):  # split across DMA queues
                nc.sync.dma_start(
                    h_t[:, kg:kg + 8, :], h16.ap()[tci, :, kg:kg + 8, :]
                )
            if split_kv:
                hkv_t = pools["hkv"].tile([128, KT, TCH], BF16, tag="hkv")
                for kg in range(0, KT, 8):
                    nc.sync.dma_start(
                        hkv_t[:, kg:kg + 8, :], h16kv.ap()[tci, :, kg:kg + 8, :]
                    )
            else:
                hkv_t = h_t
            if tci == 0:
                for kg in range(0, KT, 8):
                    nc.sync.dma_start(
                        wq_sb[:, kg:kg + 8, :], wq16.ap()[:, kg:kg + 8, :]
                    )
            cos_t = pools["cs"].tile([128, TCH], FP32, tag="cos")
            sin_t = pools["cs"].tile([128, TCH], FP32, tag="sin")
            nc.sync.dma_start(cos_t[:], cosq.ap()[:, tsl])
            nc.sync.dma_start(sin_t[:], sinq.ap()[:, tsl])
            if split_kv:
                cosk_t = pools["cs"].tile([128, TCH], FP32, tag="cosk")
                sink_t = pools["cs"].tile([128, TCH], FP32, tag="sink")
                nc.sync.dma_start(cosk_t[:], cosk.ap()[:, tsl])
                nc.sync.dma_start(sink_t[:], sink.ap()[:, tsl])
            else:
                cosk_t, sink_t = cos_t, sin_t

            # K first so the attention diagonal has it earliest
            psk = pools["proj"].tile([128, TCH], FP32, tag="proj", name=f"psk{tci}")
            for k in range(KT):
                nc.tensor.matmul(
                    psk[:], wk_sb[:, k, :], hkv_t[:, k, :],
                    start=(k == 0), stop=(k == KT - 1),
                )
            rope(psk, cosk_t, sink_t, KT_res[:, tsl])

            for g in range(GQ):
                psq = pools["proj"].tile([128, TCH], FP32, tag="proj", name=f"psq{tci}_{g}")
                for k in range(KT):
                    nc.tensor.matmul(
                        psq[:], wq_sb[:, k, g * 128:(g + 1) * 128], h_t[:, k, :],
                        start=(k == 0), stop=(k == KT - 1),
                    )
                qT[g] = pools["q"].tile(
                    [128, TCH], BF16, tag=f"q{g}", name=f"qT{tci}_{g}"
                )
                rope(psq, cos_t, sin_t, qT[g][:])

            psv = pools["proj"].tile([128, TCH], FP32, tag="proj", name=f"psv{tci}")
            for k in range(KT):
                nc.tensor.matmul(
                    psv[:], wv_sb[:, k, :], hkv_t[:, k, :],
                    start=(k == 0), stop=(k == KT - 1),
                )
            vsb = pools["vsb"].tile([128, TCH], BF16, tag="vsb")
            nc.scalar.copy(vsb[:], psv[:])
            vdr = dram_pool.tile([128, TCH], BF16, tag="vdr")
            nc.sync.dma_start(vdr[:], vsb[:])
            for tb in range(4):
                nc.sync.dma_start_transpose(
                    V_res[:, tci * 4 + tb, :], vdr[:, tb * 128:(tb + 1) * 128]
                )

        def attn_head(tci, g, qT, ao, pump):
            """attention for head g of chunk tci -> ao[:, g, :].

            Software-pipelined: score matmuls run LOOKAHEAD blocks ahead of
            the PV matmuls so the exp(ACT)->mask/es(DVE) latency is hidden;
            one output-projection quad of the previous chunk is pumped per
            step as PE filler."""
            b, it = tci // 2, tci % 2
            njt = 4 * (it + 1)
            es = pools["es"].tile([128, TCH], FP32R, tag="es")
            pv = pools["pv"].tile([128, TCH], FP32, tag="pv")
            LOOKAHEAD = 2
            exs = [None] * njt

            def score(jt):
                pss = pools["pss"].tile([128, TCH], FP32, tag="pss")
                nc.tensor.matmul(
                    pss[:],
                    KT_res[:, b * S + jt * 128: b * S + (jt + 1) * 128],
                    qT[g][:],
                    start=True, stop=True,
                )
                ex = pools["ex"].tile([128, TCH], BF16, tag="ex", name=f"ex{tci}_{g}_{jt}")
                nc.scalar.activation(
                    ex[:], pss[:], mybir.ActivationFunctionType.Exp, scale=SCALE
                )
                r = jt - 4 * it
                if r >= 0:  # diagonal block -> causal mask
                    nc.vector.tensor_mul(
                        ex[:], ex[:], mask_sb[:, r * TCH:(r + 1) * TCH]
                    )
                if jt == 0:
                    nc.vector.tensor_copy(es[:], ex[:])
                else:
                    nc.vector.tensor_add(es[:], es[:], ex[:])
                exs[jt] = ex

            for jt in range(min(LOOKAHEAD, njt)):
                score(jt)
            for jt in range(njt):
                if pump is not None:
                    # even chunks have half the jt steps but the same 32
                    # filler quads; pump 2 there so none are left to drain
                    # against the next chunk's projection phase
                    for _ in range(2 - it):
                        next(pump, None)
                if jt + LOOKAHEAD < njt:
                    score(jt + LOOKAHEAD)
                nc.tensor.matmul(
                    pv[:], V_res[:, b * 8 + jt, :], exs[jt][:],
                    start=(jt == 0), stop=(jt == njt - 1),
                )
                exs[jt] = None
            psd = pools["pss"].tile([128, TCH], FP32, tag="pss")
            nc.tensor.matmul(psd[:], ones_sb[:], es[:], start=True, stop=True)
            rec = pools["rec"].tile([128, TCH], FP32, tag="rec")
            nc.vector.reciprocal_approx_fast(rec[:], psd[:])
            nc.vector.tensor_mul(ao[:, g, :], pv[:], rec[:])

        def load_wo(tci, e):
            wo_t = pools["wo"].tile(
                [128, GQ, TCH], BF16, tag="wo", name=f"wo{tci}_{e}"
            )
            nc.sync.dma_start(wo_t[:], wo16.ap()[:, :, e * TCH:(e + 1) * TCH])
            return wo_t

        def p3_gen(tci, ao):
            """generator: each next() emits one [128-token x 512-col] output
            quad of the projection for chunk tci; pumped from inside the
            attention jt loop so the PE always has filler matmuls. wo slices
            are prefetched one e-slice ahead; PSUM is drained in halves on
            ACT and DVE so neither engine queue grows a long blocker."""
            wo_t = load_wo(tci, 0)
            for e in range(8):
                esl = slice(e * TCH, (e + 1) * TCH)
                wo_next = load_wo(tci, e + 1) if e + 1 < 8 else None
                for tb in range(4):
                    pso = pools["pso"].tile([128, TCH], FP32, tag="pso", name=f"pso{tci}_{e}_{tb}")
                    for g in range(GQ):
                        nc.tensor.matmul(
                            pso[:], ao[:, g, tb * 128:(tb + 1) * 128], wo_t[:, g, :],
                            start=(g == 0), stop=(g == GQ - 1),
                        )
                    ob = pools["ob"].tile([128, TCH], BF16, tag="ob", name=f"ob{tci}_{e}_{tb}")
                    nc.scalar.copy(ob[:, 0:TCH // 2], pso[:, 0:TCH // 2])
                    nc.vector.tensor_copy(ob[:, TCH // 2:], pso[:, TCH // 2:])
                    nc.sync.dma_start(
                        outp.ap()[tci * TCH + tb * 128: tci * TCH + (tb + 1) * 128, esl],
                        ob[:],
                    )
                    yield
                wo_t = wo_next

        prev = None  # p3 generator of previous chunk
        for tci in range(NTCH):
            qT = [None] * GQ
            p1(tci, qT)
            ao = pools["ao"].tile([128, GQ, TCH], BF16, tag="ao", name=f"ao{tci}")
            for g in range(GQ):
                attn_head(tci, g, qT, ao, prev)
                if prev is not None:
                    next(prev, None)
                    next(prev, None)
            if prev is not None:
                for _ in prev:
                    pass
            prev = p3_gen(tci, ao)
        for _ in prev:
            pass

    nc.finalize()
    return nc


def _get_program(split_kv: bool):
    if split_kv not in _PROG_CACHE:
        _PROG_CACHE[split_kv] = _build_program(split_kv)
    return _PROG_CACHE[split_kv]


def kernel(
    hidden_states, wq, wk, wv, wo, kv_cache, position_ids,
    kv_page_indices, kv_page_indptr, kv_last_page_lens, qo_indptr,
    _run_kwargs: dict | None = None,
):
    hidden_states = np.asarray(hidden_states, np.float32)
    wq = np.asarray(wq, np.float32)
    wk = np.asarray(wk, np.float32)
    wv = np.asarray(wv, np.float32)
    wo = np.asarray(wo, np.float32)
    position_ids = np.asarray(position_ids, np.int32)
    qo_indptr = np.asarray(qo_indptr, np.int64)

    nnz = hidden_states.shape[0]
    b = qo_indptr.shape[0] - 1
    assert nnz == T and b == B, (nnz, b)
    assert np.array_equal(qo_indptr, np.arange(B + 1, dtype=np.int64) * S), (
        "kernel assumes uniform sequence lengths of 1024"
    )

    # Page-gather order: the reference gathers pages in list order, so the
    # token with position p within its sequence lands at page-order rank p.
    # KV must be fed in rank order; the q path stays in token order.
    perm = np.empty(T, np.int64)
    identity = True
    for bi in range(B):
        pos_b = position_ids[bi * S:(bi + 1) * S].astype(np.int64)
        assert np.array_equal(np.sort(pos_b), np.arange(S)), (
            "kernel assumes positions cover 0..S-1 exactly once per sequence"
        )
        inv = np.empty(S, np.int64)
        inv[pos_b] = np.arange(S)
        perm[bi * S:(bi + 1) * S] = bi * S + inv
        if not np.array_equal(inv, np.arange(S)):
            identity = False

    h16 = _tile_h(hidden_states)
    cosq, sinq = _rope_tables(position_ids)
    maskd = _masks_np()
    ones = np.ones((128, 128), np.float32)

    split_kv = not identity
    nc = _get_program(split_kv)

    in_maps = []
    for c in range(NCORES):
        im = {
            "h16": h16,
            "wq16": _tile_w(np.ascontiguousarray(wq[c * DQ:(c + 1) * DQ, :].T)),
            "wk16": _tile_w(np.ascontiguousarray(wk[c * HD:(c + 1) * HD, :].T)),
            "wv16": _tile_w(np.ascontiguousarray(wv[c * HD:(c + 1) * HD, :].T)),
            "wo16": _tile_wo(np.ascontiguousarray(wo[:, c * DQ:(c + 1) * DQ].T)),
            "cosq": cosq,
            "sinq": sinq,
            "maskd": maskd,
            "onesd": ones,
        }
        if split_kv:
            im["h16kv"] = _tile_h(hidden_states[perm])
            cosk, sink = _rope_tables(position_ids[perm])
            im["cosk"] = cosk
            im["sink"] = sink
        in_maps.append(im)

    res = run_bass_kernel_spmd(
        nc, in_maps, core_ids=list(range(NCORES)), **(_run_kwargs or {})
    )
    out = np.zeros((T, HID), np.float64)
    for c in range(NCORES):
        out += res.results[c]["outp"].astype(np.float64)
    kernel.last_results = res  # type: ignore[attr-defined]
    return out.astype(np.float32)


# revision 31
# speedup vs baseline: 1.0071x; 1.0071x over previous
"""Trainium2 Bass kernel for paged-attention Llama-style block (nn_L4maAttention).

Sharding: tensor-parallel over heads across 8 NeuronCores. Core c owns
q-heads [4c, 4c+4), kv-head c, wq/wk/wv row shards and the matching wo
column shard. Each core computes a full [T, HID] partial of the output
projection (bf16); the host sums the 8 partials (the TP reduce).

v2: fused per-chunk pipeline in bf16. For each 512-token chunk:
  P1: QKV projections (bf16 matmuls, fp32 PSUM) + Llama-3.1 RoPE on Q/K
      (fused halfswap via partition-shifted DVE muls) + V transpose via
      DRAM-roundtrip XBAR DMA. PSUM accumulators rotate through a
      3-bank pool so the PE never stalls on drains.
  A:  causal attention with transposed scores [k on partitions]; exp on
      ACT (bf16 out); causal mask-mul on DVE (bf16 2x); denominator
      accumulated on GpSimd (fp32), reduced via a ones-matmul into the
      recycled score-PSUM pool; reciprocal_approx_fast on DVE.
  P3: output projection partial; PSUM drained to bf16 alternating
      ACT/DVE; P3 of the previous chunk is interleaved between attention
      heads to fill PE bubbles.
The fused stream keeps the PE continuously busy so the HAM activity
throttle stays at full speed (the v1 kernel lost ~35% of its runtime to
k=4/8 half-throttle windows at phase boundaries).
"""

import math
import sys
from contextlib import ExitStack

import numpy as np

for _p in ("/opt/trn_rl_repo",):
    if _p not in sys.path:
        sys.path.insert(0, _p)

import ml_dtypes  # noqa: E402

import concourse.mybir as mybir  # noqa: E402
import concourse.tile as tile  # noqa: E402
from concourse import bacc  # noqa: E402
from concourse.bass_utils import run_bass_kernel_spmd  # noqa: E402

NCORES = 8
HID = 4096
NH = 32
NKV = 8
HD = 128
B = 4
S = 1024
T = B * S
GQ = NH // NCORES          # q heads per core = 4
DQ = GQ * HD               # 512
TCH = 512                  # token chunk
NTCH = T // TCH            # 8
KT = HID // 128            # 32 contraction tiles
SCALE = 1.0 / math.sqrt(HD)

FP32 = mybir.dt.float32
FP32R = mybir.dt.float32r
BF16 = mybir.dt.bfloat16
NP_BF16 = ml_dtypes.bfloat16

_PROG_CACHE: dict = {}


def _llama31_freqs_np(head_dim: int) -> np.ndarray:
    half = head_dim // 2
    theta, scale, low_ff, high_ff, old_ctx = 500000.0, 8.0, 1.0, 4.0, 8192.0
    freq = 1.0 / (theta ** (np.arange(half, dtype=np.float64) * 2.0 / head_dim))
    wavelen = 2.0 * np.pi / freq
    low_wl, high_wl = old_ctx / low_ff, old_ctx / high_ff
    smooth = (old_ctx / wavelen - low_ff) / (high_ff - low_ff)
    out = np.where(
        wavelen < high_wl,
        freq,
        np.where(wavelen > low_wl, freq / scale, (1.0 - smooth) * freq / scale + smooth * freq),
    )
    return out.astype(np.float64)


def _rope_tables(pos: np.ndarray) -> tuple[np.ndarray, np.ndarray]:
    """cosF [128, n]: cos duplicated on both partition halves.
    sinF2 [128, n]: -sin on rows 0-63, +sin on rows 64-127. The kernel
    computes out = x*cosF + t2 with t2[0:64] = x[64:128]*sinF2[0:64] and
    t2[64:128] = x[0:64]*sinF2[64:128], which equals rotate-half RoPE."""
    freqs = _llama31_freqs_np(HD)
    ang = pos.astype(np.float64)[None, :] * freqs[:, None]  # [64, n]
    c = np.cos(ang).astype(np.float32)
    s = np.sin(ang).astype(np.float32)
    cosF = np.concatenate([c, c], axis=0)
    sinF2 = np.concatenate([-s, s], axis=0)
    return np.ascontiguousarray(cosF), np.ascontiguousarray(sinF2)


def _masks_np() -> np.ndarray:
    """4 diagonal-block masks [128, 4*512]: mask[r][j, i] = (128*r + j) <= i."""
    m = np.zeros((128, 4 * TCH), np.float32)
    j = np.arange(128)[:, None]
    i = np.arange(TCH)[None, :]
    for r in range(4):
        m[:, r * TCH:(r + 1) * TCH] = ((128 * r + j) <= i).astype(np.float32)
    return m.astype(NP_BF16)


def _tile_h(h: np.ndarray) -> np.ndarray:
    """[T, HID] fp32 -> [NTCH, 128, KT, TCH] bf16 with
    out[tci, p, k, t] = h[tci*TCH + t, k*128 + p]."""
    a = h.reshape(NTCH, TCH, KT, 128).transpose(0, 3, 2, 1)
    return np.ascontiguousarray(a.astype(NP_BF16))


def _tile_w(wT: np.ndarray) -> np.ndarray:
    """[HID, D] fp32 -> [128, KT, D] bf16 with out[p, k, d] = wT[k*128+p, d]."""
    a = wT.reshape(KT, 128, -1).transpose(1, 0, 2)
    return np.ascontiguousarray(a.astype(NP_BF16))


def _tile_wo(woT: np.ndarray) -> np.ndarray:
    """[DQ, HID] fp32 -> [128, GQ, HID] bf16 with out[p, g, e] = woT[g*128+p, e]."""
    a = woT.reshape(GQ, 128, HID).transpose(1, 0, 2)
    return np.ascontiguousarray(a.astype(NP_BF16))


def _build_program(split_kv: bool):
    nc = bacc.Bacc(
        "TRN2",
        target_bir_lowering=False,
        debug=False,
        enable_asserts=False,
        num_devices=NCORES,
    )
    h16 = nc.dram_tensor("h16", [NTCH, 128, KT, TCH], BF16, kind="ExternalInput")
    h16kv = (
        nc.dram_tensor("h16kv", [NTCH, 128, KT, TCH], BF16, kind="ExternalInput")
        if split_kv
        else h16
    )
    wq16 = nc.dram_tensor("wq16", [128, KT, DQ], BF16, kind="ExternalInput")
    wk16 = nc.dram_tensor("wk16", [128, KT, HD], BF16, kind="ExternalInput")
    wv16 = nc.dram_tensor("wv16", [128, KT, HD], BF16, kind="ExternalInput")
    wo16 = nc.dram_tensor("wo16", [128, GQ, HID], BF16, kind="ExternalInput")
    cosq = nc.dram_tensor("cosq", [128, T], FP32, kind="ExternalInput")
    sinq = nc.dram_tensor("sinq", [128, T], FP32, kind="ExternalInput")
    if split_kv:
        cosk = nc.dram_tensor("cosk", [128, T], FP32, kind="ExternalInput")
        sink = nc.dram_tensor("sink", [128, T], FP32, kind="ExternalInput")
    else:
        cosk, sink = cosq, sinq
    maskd = nc.dram_tensor("maskd", [128, 4 * TCH], BF16, kind="ExternalInput")
    onesd = nc.dram_tensor("onesd", [128, 128], FP32R, kind="ExternalInput")
    outp = nc.dram_tensor("outp", [T, HID], BF16, kind="ExternalOutput")

    with tile.TileContext(nc) as tc, ExitStack() as ctx:
        const_pool = ctx.enter_context(tc.tile_pool(name="const", bufs=1))
        dram_pool = ctx.enter_context(tc.tile_pool(name="dram", bufs=2, space="DRAM"))
        res_pool = ctx.enter_context(tc.tile_pool(name="res", bufs=1))

        ones_sb = const_pool.tile([128, 128], FP32R)
        nc.sync.dma_start(ones_sb[:], onesd.ap()[:, :])
        mask_sb = const_pool.tile([128, 4 * TCH], BF16)
        nc.sync.dma_start(mask_sb[:], maskd.ap()[:, :])

        # resident weights; DMAs are emitted inside p1(0) after the first h
        # chunk so the startup critical path (h0 + wk) isn't queued behind wq
        wq_sb = res_pool.tile([128, KT, DQ], BF16)
        wk_sb = res_pool.tile([128, KT, HD], BF16)
        wv_sb = res_pool.tile([128, KT, HD], BF16)



        KT_res = res_pool.tile([128, T], BF16)          # 8KB/part
        V_res = res_pool.tile([128, KT, HD], BF16)      # 8KB/part

        hb = 1 if split_kv else 2
        pools = {
            "h": ctx.enter_context(tc.tile_pool(name="h", bufs=hb)),
            "cs": ctx.enter_context(tc.tile_pool(name="cs", bufs=2)),
            "q": ctx.enter_context(tc.tile_pool(name="q", bufs=2)),
            "t": ctx.enter_context(tc.tile_pool(name="t", bufs=1)),
            "vsb": ctx.enter_context(tc.tile_pool(name="vsb", bufs=2)),
            "ex": ctx.enter_context(tc.tile_pool(name="ex", bufs=5)),
            "es": ctx.enter_context(tc.tile_pool(name="es", bufs=2)),
            "rec": ctx.enter_context(tc.tile_pool(name="rec", bufs=2)),
            "ao": ctx.enter_context(tc.tile_pool(name="ao", bufs=2)),
            "wo": ctx.enter_context(tc.tile_pool(name="wo", bufs=3)),
            "ob": ctx.enter_context(tc.tile_pool(name="ob", bufs=4)),
            "proj": ctx.enter_context(tc.tile_pool(name="proj", bufs=2, space="PSUM")),
            "pss": ctx.enter_context(tc.tile_pool(name="pss", bufs=2, space="PSUM")),
            "pv": ctx.enter_context(tc.tile_pool(name="pv", bufs=1, space="PSUM")),
            "pso": ctx.enter_context(tc.tile_pool(name="pso", bufs=3, space="PSUM")),
        }
        if split_kv:
            pools["hkv"] = ctx.enter_context(tc.tile_pool(name="hkv", bufs=1))

        def rope(ps, cos_t, sin_t, out_ap):
            # out[0:64] = x1*cos - x2*sin, out[64:128] = x2*cos + x1*sin via
            # partition-shifted muls (no DMA in the rope dependency chain)
            t1 = pools["t"].tile([128, TCH], FP32, tag="t1")
            t2 = pools["t"].tile([128, TCH], FP32, tag="t2")
            nc.vector.tensor_mul(t1[:], ps[:], cos_t[:])
            nc.vector.tensor_mul(t2[0:64, :], ps[64:128, :], sin_t[0:64, :])
            nc.vector.tensor_mul(t2[64:128, :], ps[0:64, :], sin_t[64:128, :])
            nc.vector.tensor_add(out_ap, t1[:], t2[:])

        def p1(tci, qT):
            """projections + rope for chunk tci; writes qT[g], KT_res, V_res."""
            tsl = slice(tci * TCH, (tci + 1) * TCH)
            if tci == 0:
                # K/V-projection weights first: chunk 0's first matmuls need
                # wk + the first h quarter, so those lead the DMA queue
                nc.sync.dma_start(wk_sb[:], wk16.ap()[:, :, :])
                nc.sync.dma_start(wv_sb[:], wv16.ap()[:, :, :])
            h_t = pools["h"].tile([128, KT, TCH], BF16, tag="h")
            for kg in range(0, KT, 8):  # split across DMA queues
                nc.sync.dma_start(
                    h_t[:, kg:kg + 8, :], h16.ap()[tci, :, kg:kg + 8, :]
                )
            if split_kv:
                hkv_t = pools["hkv"].tile([128, KT, TCH], BF16, tag="hkv")
                for kg in range(0, KT, 8):
                    nc.sync.dma_start(
                        hkv_t[:, kg:kg + 8, :], h16kv.ap()[tci, :, kg:kg + 8, :]
                    )
            else:
                hkv_t = h_t
            if tci == 0:
                for kg in range(0, KT, 8):
                    nc.sync.dma_start(
                        wq_sb[:, kg:kg + 8, :], wq16.ap()[:, kg:kg + 8, :]
                    )
            cos_t = pools["cs"].tile([128, TCH], FP32, tag="cos")
            sin_t = pools["cs"].tile([128, TCH], FP32, tag="sin")
            nc.sync.dma_start(cos_t[:], cosq.ap()[:, tsl])
            nc.sync.dma_start(sin_t[:], sinq.ap()[:, tsl])
            if split_kv:
                cosk_t = pools["cs"].tile([128, TCH], FP32, tag="cosk")
                sink_t = pools["cs"].tile([128, TCH], FP32, tag="sink")
                nc.sync.dma_start(cosk_t[:], cosk.ap()[:, tsl])
                nc.sync.dma_start(sink_t[:], sink.ap()[:, tsl])
            else:
                cosk_t, sink_t = cos_t, sin_t

            # K first so the attention diagonal has it earliest
            psk = pools["proj"].tile([128, TCH], FP32, tag="proj", name=f"psk{tci}")
            for k in range(KT):
                nc.tensor.matmul(
                    psk[:], wk_sb[:, k, :], hkv_t[:, k, :],
                    start=(k == 0), stop=(k == KT - 1),
                )
            rope(psk, cosk_t, sink_t, KT_res[:, tsl])

            for g in range(GQ):
                psq = pools["proj"].tile([128, TCH], FP32, tag="proj", name=f"psq{tci}_{g}")
                for k in range(KT):
                    nc.tensor.matmul(
                        psq[:], wq_sb[:, k, g * 128:(g + 1) * 128], h_t[:, k, :],
                        start=(k == 0), stop=(k == KT - 1),
                    )
                qT[g] = pools["q"].tile(
                    [128, TCH], BF16, tag=f"q{g}", name=f"qT{tci}_{g}"
                )
                rope(psq, cos_t, sin_t, qT[g][:])

            psv = pools["proj"].tile([128, TCH], FP32, tag="proj", name=f"psv{tci}")
            for k in range(KT):
                nc.tensor.matmul(
                    psv[:], wv_sb[:, k, :], hkv_t[:, k, :],
                    start=(k == 0), stop=(k == KT - 1),
                )
            vsb = pools["vsb"].tile([128, TCH], BF16, tag="vsb")
            nc.scalar.copy(vsb[:], psv[:])
            vdr = dram_pool.tile([128, TCH], BF16, tag="vdr")
            nc.sync.dma_start(vdr[:], vsb[:])
            for tb in range(4):
                nc.sync.dma_start_transpose(
                    V_res[:, tci * 4 + tb, :], vdr[:, tb * 128:(tb + 1) * 128]
                )

        def attn_head(tci, g, qT, ao, pump):
            """attention for head g of chunk tci -> ao[:, g, :].

            Software-pipelined: score matmuls run LOOKAHEAD blocks ahead of
            the PV matmuls so the exp(ACT)->mask/es(DVE) latency is hidden;
            one output-projection quad of the previous chunk is pumped per
            step as PE filler."""
            b, it = tci // 2, tci % 2
            njt = 4 * (it + 1)
            es = pools["es"].tile([128, TCH], FP32R, tag="es")
            pv = pools["pv"].tile([128, TCH], FP32, tag="pv")
            LOOKAHEAD = 2
            exs = [None] * njt

            def score(jt):
                pss = pools["pss"].tile([128, TCH], FP32, tag="pss")
                nc.tensor.matmul(
                    pss[:],
                    KT_res[:, b * S + jt * 128: b * S + (jt + 1) * 128],
                    qT[g][:],
                    start=True, stop=True,
                )
                ex = pools["ex"].tile([128, TCH], BF16, tag="ex", name=f"ex{tci}_{g}_{jt}")
                nc.scalar.activation(
                    ex[:], pss[:], mybir.ActivationFunctionType.Exp, scale=SCALE
                )
                r = jt - 4 * it
                if r >= 0:  # diagonal block -> causal mask
                    nc.vector.tensor_mul(
                        ex[:], ex[:], mask_sb[:, r * TCH:(r + 1) * TCH]
                    )
                if jt == 0:
                    nc.vector.tensor_copy(es[:], ex[:])
                else:
                    nc.vector.tensor_add(es[:], es[:], ex[:])
                exs[jt] = ex

            for jt in range(min(LOOKAHEAD, njt)):
                score(jt)
            for jt in range(njt):
                if pump is not None:
                    next(pump, None)
                if jt + LOOKAHEAD < njt:
                    score(jt + LOOKAHEAD)
                nc.tensor.matmul(
                    pv[:], V_res[:, b * 8 + jt, :], exs[jt][:],
                    start=(jt == 0), stop=(jt == njt - 1),
                )
                exs[jt] = None
            psd = pools["pss"].tile([128, TCH], FP32, tag="pss")
            nc.tensor.matmul(psd[:], ones_sb[:], es[:], start=True, stop=True)
            rec = pools["rec"].tile([128, TCH], FP32, tag="rec")
            nc.vector.reciprocal_approx_fast(rec[:], psd[:])
            nc.vector.tensor_mul(ao[:, g, :], pv[:], rec[:])

        def load_wo(tci, e):
            wo_t = pools["wo"].tile(
                [128, GQ, TCH], BF16, tag="wo", name=f"wo{tci}_{e}"
            )
            with tc.high_priority():
                nc.sync.dma_start(wo_t[:], wo16.ap()[:, :, e * TCH:(e + 1) * TCH])
            return wo_t

        def p3_gen(tci, ao):
            """generator: each next() emits one [128-token x 512-col] output
            quad of the projection for chunk tci; pumped from inside the
            attention jt loop so the PE always has filler matmuls. wo slices
            are prefetched one e-slice ahead; PSUM is drained in halves on
            ACT and DVE so neither engine queue grows a long blocker."""
            wo_t = load_wo(tci, 0)
            for e in range(8):
                esl = slice(e * TCH, (e + 1) * TCH)
                wo_next = load_wo(tci, e + 1) if e + 1 < 8 else None
                for tb in range(4):
                    pso = pools["pso"].tile([128, TCH], FP32, tag="pso", name=f"pso{tci}_{e}_{tb}")
                    for g in range(GQ):
                        nc.tensor.matmul(
                            pso[:], ao[:, g, tb * 128:(tb + 1) * 128], wo_t[:, g, :],
                            start=(g == 0), stop=(g == GQ - 1),
                        )
                    ob = pools["ob"].tile([128, TCH], BF16, tag="ob", name=f"ob{tci}_{e}_{tb}")
                    # high priority: the moment a quad completes, its drain
                    # should win the next ACT/DVE slot so the PSUM bank
                    # recycles before the filler pipeline needs it again
                    with tc.high_priority():
                        nc.scalar.copy(ob[:, 0:TCH // 2], pso[:, 0:TCH // 2])
                        nc.vector.tensor_copy(ob[:, TCH // 2:], pso[:, TCH // 2:])
                    nc.sync.dma_start(
                        outp.ap()[tci * TCH + tb * 128: tci * TCH + (tb + 1) * 128, esl],
                        ob[:],
                    )
                    yield
                wo_t = wo_next

        prev = None  # p3 generator of previous chunk
        for tci in range(NTCH):
            qT = [None] * GQ
            p1(tci, qT)
            ao = pools["ao"].tile([128, GQ, TCH], BF16, tag="ao", name=f"ao{tci}")
            for g in range(GQ):
                attn_head(tci, g, qT, ao, prev)
                if prev is not None:
                    next(prev, None)
                    next(prev, None)
            if prev is not None:
                for _ in prev:
                    pass
            prev = p3_gen(tci, ao)
        for _ in prev:
            pass

    nc.finalize()
    return nc


def _get_program(split_kv: bool):
    if split_kv not in _PROG_CACHE:
        _PROG_CACHE[split_kv] = _build_program(split_kv)
    return _PROG_CACHE[split_kv]


def kernel(
    hidden_states, wq, wk, wv, wo, kv_cache, position_ids,
    kv_page_indices, kv_page_indptr, kv_last_page_lens, qo_indptr,
    _run_kwargs: dict | None = None,
):
    hidden_states = np.asarray(hidden_states, np.float32)
    wq = np.asarray(wq, np.float32)
    wk = np.asarray(wk, np.float32)
    wv = np.asarray(wv, np.float32)
    wo = np.asarray(wo, np.float32)
    position_ids = np.asarray(position_ids, np.int32)
    qo_indptr = np.asarray(qo_indptr, np.int64)

    nnz = hidden_states.shape[0]
    b = qo_indptr.shape[0] - 1
    assert nnz == T and b == B, (nnz, b)
    assert np.array_equal(qo_indptr, np.arange(B + 1, dtype=np.int64) * S), (
        "kernel assumes uniform sequence lengths of 1024"
    )

    # Page-gather order: the reference gathers pages in list order, so the
    # token with position p within its sequence lands at page-order rank p.
    # KV must be fed in rank order; the q path stays in token order.
    perm = np.empty(T, np.int64)
    identity = True
    for bi in range(B):
        pos_b = position_ids[bi * S:(bi + 1) * S].astype(np.int64)
        assert np.array_equal(np.sort(pos_b), np.arange(S)), (
            "kernel assumes positions cover 0..S-1 exactly once per sequence"
        )
        inv = np.empty(S, np.int64)
        inv[pos_b] = np.arange(S)
        perm[bi * S:(bi + 1) * S] = bi * S + inv
        if not np.array_equal(inv, np.arange(S)):
            identity = False

    h16 = _tile_h(hidden_states)
    cosq, sinq = _rope_tables(position_ids)
    maskd = _masks_np()
    ones = np.ones((128, 128), np.float32)

    split_kv = not identity
    nc = _get_program(split_kv)

    in_maps = []
    for c in range(NCORES):
        im = {
            "h16": h16,
            "wq16": _tile_w(np.ascontiguousarray(wq[c * DQ:(c + 1) * DQ, :].T)),
            "wk16": _tile_w(np.ascontiguousarray(wk[c * HD:(c + 1) * HD, :].T)),
            "wv16": _tile_w(np.ascontiguousarray(wv[c * HD:(c + 1) * HD, :].T)),
            "wo16": _tile_wo(np.ascontiguousarray(wo[:, c * DQ:(c + 1) * DQ].T)),
            "cosq": cosq,
            "sinq": sinq,
            "maskd": maskd,
            "onesd": ones,
        }
        if split_kv:
            im["h16kv"] = _tile_h(hidden_states[perm])
            cosk, sink = _rope_tables(position_ids[perm])
            im["cosk"] = cosk
            im["sink"] = sink
        in_maps.append(im)

    res = run_bass_kernel_spmd(
        nc, in_maps, core_ids=list(range(NCORES)), **(_run_kwargs or {})
    )
    out = np.zeros((T, HID), np.float64)
    for c in range(NCORES):
        out += res.results[c]["outp"].astype(np.float64)
    kernel.last_results = res  # type: ignore[attr-defined]
    return out.astype(np.float32)


# revision 34
# speedup vs baseline: 1.0545x; 1.0471x over previous
"""Trainium2 Bass kernel for paged-attention Llama-style block (nn_L4maAttention).

Sharding: tensor-parallel over heads across 8 NeuronCores. Core c owns
q-heads [4c, 4c+4), kv-head c, wq/wk/wv row shards and the matching wo
column shard. Each core computes a full [T, HID] partial of the output
projection (bf16); the host sums the 8 partials (the TP reduce).

v2: fused per-chunk pipeline in bf16. For each 512-token chunk:
  P1: QKV projections (bf16 matmuls, fp32 PSUM) + Llama-3.1 RoPE on Q/K
      (fused halfswap via partition-shifted DVE muls) + V transpose via
      DRAM-roundtrip XBAR DMA. PSUM accumulators rotate through a
      3-bank pool so the PE never stalls on drains.
  A:  causal attention with transposed scores [k on partitions]; exp on
      ACT (bf16 out); causal mask-mul on DVE (bf16 2x); denominator
      accumulated on GpSimd (fp32), reduced via a ones-matmul into the
      recycled score-PSUM pool; reciprocal_approx_fast on DVE.
  P3: output projection partial; PSUM drained to bf16 alternating
      ACT/DVE; P3 of the previous chunk is interleaved between attention
      heads to fill PE bubbles.
The fused stream keeps the PE continuously busy so the HAM activity
throttle stays at full speed (the v1 kernel lost ~35% of its runtime to
k=4/8 half-throttle windows at phase boundaries).
"""

import math
import sys
from contextlib import ExitStack

import numpy as np

for _p in ("/opt/trn_rl_repo",):
    if _p not in sys.path:
        sys.path.insert(0, _p)

import ml_dtypes  # noqa: E402

import concourse.mybir as mybir  # noqa: E402
import concourse.tile as tile  # noqa: E402
from concourse import bacc  # noqa: E402
from concourse.bass_utils import run_bass_kernel_spmd  # noqa: E402

NCORES = 8
HID = 4096
NH = 32
NKV = 8
HD = 128
B = 4
S = 1024
T = B * S
GQ = NH // NCORES          # q heads per core = 4
DQ = GQ * HD               # 512
TCH = 512                  # token chunk
NTCH = T // TCH            # 8
KT = HID // 128            # 32 contraction tiles
SCALE = 1.0 / math.sqrt(HD)

FP32 = mybir.dt.float32
FP32R = mybir.dt.float32r
BF16 = mybir.dt.bfloat16
NP_BF16 = ml_dtypes.bfloat16

_PROG_CACHE: dict = {}


def _llama31_freqs_np(head_dim: int) -> np.ndarray:
    half = head_dim // 2
    theta, scale, low_ff, high_ff, old_ctx = 500000.0, 8.0, 1.0, 4.0, 8192.0
    freq = 1.0 / (theta ** (np.arange(half, dtype=np.float64) * 2.0 / head_dim))
    wavelen = 2.0 * np.pi / freq
    low_wl, high_wl = old_ctx / low_ff, old_ctx / high_ff
    smooth = (old_ctx / wavelen - low_ff) / (high_ff - low_ff)
    out = np.where(
        wavelen < high_wl,
        freq,
        np.where(wavelen > low_wl, freq / scale, (1.0 - smooth) * freq / scale + smooth * freq),
    )
    return out.astype(np.float64)


def _rope_tables(pos: np.ndarray) -> tuple[np.ndarray, np.ndarray]:
    """cosF [128, n]: cos duplicated on both partition halves.
    sinF2 [128, n]: -sin on rows 0-63, +sin on rows 64-127. The kernel
    computes out = x*cosF + t2 with t2[0:64] = x[64:128]*sinF2[0:64] and
    t2[64:128] = x[0:64]*sinF2[64:128], which equals rotate-half RoPE."""
    freqs = _llama31_freqs_np(HD)
    ang = pos.astype(np.float64)[None, :] * freqs[:, None]  # [64, n]
    c = np.cos(ang).astype(np.float32)
    s = np.sin(ang).astype(np.float32)
    cosF = np.concatenate([c, c], axis=0)
    sinF2 = np.concatenate([-s, s], axis=0)
    return np.ascontiguousarray(cosF), np.ascontiguousarray(sinF2)


def _masks_np() -> np.ndarray:
    """4 diagonal-block masks [128, 4*512]: mask[r][j, i] = (128*r + j) <= i."""
    m = np.zeros((128, 4 * TCH), np.float32)
    j = np.arange(128)[:, None]
    i = np.arange(TCH)[None, :]
    for r in range(4):
        m[:, r * TCH:(r + 1) * TCH] = ((128 * r + j) <= i).astype(np.float32)
    return m.astype(NP_BF16)


def _tile_h(h: np.ndarray) -> np.ndarray:
    """[T, HID] fp32 -> [NTCH, 128, KT, TCH] bf16 with
    out[tci, p, k, t] = h[tci*TCH + t, k*128 + p]."""
    a = h.reshape(NTCH, TCH, KT, 128).transpose(0, 3, 2, 1)
    return np.ascontiguousarray(a.astype(NP_BF16))


def _tile_w(wT: np.ndarray) -> np.ndarray:
    """[HID, D] fp32 -> [128, KT, D] bf16 with out[p, k, d] = wT[k*128+p, d]."""
    a = wT.reshape(KT, 128, -1).transpose(1, 0, 2)
    return np.ascontiguousarray(a.astype(NP_BF16))


def _tile_wo(woT: np.ndarray) -> np.ndarray:
    """[DQ, HID] fp32 -> [128, GQ, HID] bf16 with out[p, g, e] = woT[g*128+p, e]."""
    a = woT.reshape(GQ, 128, HID).transpose(1, 0, 2)
    return np.ascontiguousarray(a.astype(NP_BF16))


def _build_program(split_kv: bool):
    nc = bacc.Bacc(
        "TRN2",
        target_bir_lowering=False,
        debug=False,
        enable_asserts=False,
        num_devices=NCORES,
    )
    h16 = nc.dram_tensor("h16", [NTCH, 128, KT, TCH], BF16, kind="ExternalInput")
    h16kv = (
        nc.dram_tensor("h16kv", [NTCH, 128, KT, TCH], BF16, kind="ExternalInput")
        if split_kv
        else h16
    )
    wq16 = nc.dram_tensor("wq16", [128, KT, DQ], BF16, kind="ExternalInput")
    wk16 = nc.dram_tensor("wk16", [128, KT, HD], BF16, kind="ExternalInput")
    wv16 = nc.dram_tensor("wv16", [128, KT, HD], BF16, kind="ExternalInput")
    wo16 = nc.dram_tensor("wo16", [128, GQ, HID], BF16, kind="ExternalInput")
    cosq = nc.dram_tensor("cosq", [128, T], FP32, kind="ExternalInput")
    sinq = nc.dram_tensor("sinq", [128, T], FP32, kind="ExternalInput")
    if split_kv:
        cosk = nc.dram_tensor("cosk", [128, T], FP32, kind="ExternalInput")
        sink = nc.dram_tensor("sink", [128, T], FP32, kind="ExternalInput")
    else:
        cosk, sink = cosq, sinq
    maskd = nc.dram_tensor("maskd", [128, 4 * TCH], BF16, kind="ExternalInput")
    onesd = nc.dram_tensor("onesd", [128, 128], FP32R, kind="ExternalInput")
    outp = nc.dram_tensor("outp", [T, HID], BF16, kind="ExternalOutput")

    with tile.TileContext(nc) as tc, ExitStack() as ctx:
        const_pool = ctx.enter_context(tc.tile_pool(name="const", bufs=1))
        dram_pool = ctx.enter_context(tc.tile_pool(name="dram", bufs=2, space="DRAM"))
        res_pool = ctx.enter_context(tc.tile_pool(name="res", bufs=1))

        ones_sb = const_pool.tile([128, 128], FP32R)
        nc.sync.dma_start(ones_sb[:], onesd.ap()[:, :])
        mask_sb = const_pool.tile([128, 4 * TCH], BF16)
        nc.sync.dma_start(mask_sb[:], maskd.ap()[:, :])

        # resident weights; DMAs are emitted inside p1(0) after the first h
        # chunk so the startup critical path (h0 + wk) isn't queued behind wq
        wq_sb = res_pool.tile([128, KT, DQ], BF16)
        wk_sb = res_pool.tile([128, KT, HD], BF16)
        wv_sb = res_pool.tile([128, KT, HD], BF16)



        KT_res = res_pool.tile([128, T], BF16)          # 8KB/part
        V_res = res_pool.tile([128, KT, HD], BF16)      # 8KB/part

        hb = 1 if split_kv else 2
        pools = {
            "h": ctx.enter_context(tc.tile_pool(name="h", bufs=hb)),
            "cs": ctx.enter_context(tc.tile_pool(name="cs", bufs=3)),
            "q": ctx.enter_context(tc.tile_pool(name="q", bufs=2)),
            "t": ctx.enter_context(tc.tile_pool(name="t", bufs=2)),
            "vsb": ctx.enter_context(tc.tile_pool(name="vsb", bufs=2)),
            "ex": ctx.enter_context(tc.tile_pool(name="ex", bufs=5)),
            "es": ctx.enter_context(tc.tile_pool(name="es", bufs=3)),
            "rec": ctx.enter_context(tc.tile_pool(name="rec", bufs=2)),
            "ao": ctx.enter_context(tc.tile_pool(name="ao", bufs=2)),
            "wo": ctx.enter_context(tc.tile_pool(name="wo", bufs=3)),
            "ob": ctx.enter_context(tc.tile_pool(name="ob", bufs=4)),
            "proj": ctx.enter_context(tc.tile_pool(name="proj", bufs=2, space="PSUM")),
            "pss": ctx.enter_context(tc.tile_pool(name="pss", bufs=2, space="PSUM")),
            "pv": ctx.enter_context(tc.tile_pool(name="pv", bufs=1, space="PSUM")),
            "pso": ctx.enter_context(tc.tile_pool(name="pso", bufs=3, space="PSUM")),
        }
        if split_kv:
            pools["hkv"] = ctx.enter_context(tc.tile_pool(name="hkv", bufs=1))

        def rope(ps, cos_t, sin_t, out_ap):
            # out[0:64] = x1*cos - x2*sin, out[64:128] = x2*cos + x1*sin via
            # partition-shifted muls (no DMA in the rope dependency chain)
            t1 = pools["t"].tile([128, TCH], FP32, tag="t1")
            t2 = pools["t"].tile([128, TCH], FP32, tag="t2")
            nc.vector.tensor_mul(t1[:], ps[:], cos_t[:])
            nc.vector.tensor_mul(t2[0:64, :], ps[64:128, :], sin_t[0:64, :])
            nc.vector.tensor_mul(t2[64:128, :], ps[0:64, :], sin_t[64:128, :])
            nc.vector.tensor_add(out_ap, t1[:], t2[:])

        def p1(tci, qT):
            """projections + rope for chunk tci; writes qT[g], KT_res, V_res."""
            tsl = slice(tci * TCH, (tci + 1) * TCH)
            if tci == 0:
                # K/V-projection weights first: chunk 0's first matmuls need
                # wk + the first h quarter, so those lead the DMA queue
                nc.sync.dma_start(wk_sb[:], wk16.ap()[:, :, :])
                nc.sync.dma_start(wv_sb[:], wv16.ap()[:, :, :])
            h_t = pools["h"].tile([128, KT, TCH], BF16, tag="h")
            for kg in range(0, KT, 8):  # split across DMA queues
                nc.sync.dma_start(
                    h_t[:, kg:kg + 8, :], h16.ap()[tci, :, kg:kg + 8, :]
                )
            if split_kv:
                hkv_t = pools["hkv"].tile([128, KT, TCH], BF16, tag="hkv")
                for kg in range(0, KT, 8):
                    nc.sync.dma_start(
                        hkv_t[:, kg:kg + 8, :], h16kv.ap()[tci, :, kg:kg + 8, :]
                    )
            else:
                hkv_t = h_t
            if tci == 0:
                for kg in range(0, KT, 8):
                    nc.sync.dma_start(
                        wq_sb[:, kg:kg + 8, :], wq16.ap()[:, kg:kg + 8, :]
                    )
            cos_t = pools["cs"].tile([128, TCH], FP32, tag="cos")
            sin_t = pools["cs"].tile([128, TCH], FP32, tag="sin")
            nc.sync.dma_start(cos_t[:], cosq.ap()[:, tsl])
            nc.sync.dma_start(sin_t[:], sinq.ap()[:, tsl])
            if split_kv:
                cosk_t = pools["cs"].tile([128, TCH], FP32, tag="cosk")
                sink_t = pools["cs"].tile([128, TCH], FP32, tag="sink")
                nc.sync.dma_start(cosk_t[:], cosk.ap()[:, tsl])
                nc.sync.dma_start(sink_t[:], sink.ap()[:, tsl])
            else:
                cosk_t, sink_t = cos_t, sin_t

            # K first so the attention diagonal has it earliest
            psk = pools["proj"].tile([128, TCH], FP32, tag="proj", name=f"psk{tci}")
            for k in range(KT):
                nc.tensor.matmul(
                    psk[:], wk_sb[:, k, :], hkv_t[:, k, :],
                    start=(k == 0), stop=(k == KT - 1),
                )
            rope(psk, cosk_t, sink_t, KT_res[:, tsl])

            for g in range(GQ):
                psq = pools["proj"].tile([128, TCH], FP32, tag="proj", name=f"psq{tci}_{g}")
                for k in range(KT):
                    nc.tensor.matmul(
                        psq[:], wq_sb[:, k, g * 128:(g + 1) * 128], h_t[:, k, :],
                        start=(k == 0), stop=(k == KT - 1),
                    )
                qT[g] = pools["q"].tile(
                    [128, TCH], BF16, tag=f"q{g}", name=f"qT{tci}_{g}"
                )
                rope(psq, cos_t, sin_t, qT[g][:])

            psv = pools["proj"].tile([128, TCH], FP32, tag="proj", name=f"psv{tci}")
            for k in range(KT):
                nc.tensor.matmul(
                    psv[:], wv_sb[:, k, :], hkv_t[:, k, :],
                    start=(k == 0), stop=(k == KT - 1),
                )
            vsb = pools["vsb"].tile([128, TCH], BF16, tag="vsb")
            nc.scalar.copy(vsb[:], psv[:])
            vdr = dram_pool.tile([128, TCH], BF16, tag="vdr")
            nc.sync.dma_start(vdr[:], vsb[:])
            for tb in range(4):
                nc.sync.dma_start_transpose(
                    V_res[:, tci * 4 + tb, :], vdr[:, tb * 128:(tb + 1) * 128]
                )

        def attn_head(tci, g, qT, ao, pump):
            """attention for head g of chunk tci -> ao[:, g, :].

            Software-pipelined: score matmuls run LOOKAHEAD blocks ahead of
            the PV matmuls so the exp(ACT)->mask/es(DVE) latency is hidden;
            one output-projection quad of the previous chunk is pumped per
            step as PE filler."""
            b, it = tci // 2, tci % 2
            njt = 4 * (it + 1)
            es = pools["es"].tile([128, TCH], FP32R, tag="es")
            pv = pools["pv"].tile([128, TCH], FP32, tag="pv")
            LOOKAHEAD = 2
            exs = [None] * njt

            def score(jt):
                pss = pools["pss"].tile([128, TCH], FP32, tag="pss")
                nc.tensor.matmul(
                    pss[:],
                    KT_res[:, b * S + jt * 128: b * S + (jt + 1) * 128],
                    qT[g][:],
                    start=True, stop=True,
                )
                ex = pools["ex"].tile([128, TCH], BF16, tag="ex", name=f"ex{tci}_{g}_{jt}")
                nc.scalar.activation(
                    ex[:], pss[:], mybir.ActivationFunctionType.Exp, scale=SCALE
                )
                r = jt - 4 * it
                if r >= 0:  # diagonal block -> causal mask
                    nc.vector.tensor_mul(
                        ex[:], ex[:], mask_sb[:, r * TCH:(r + 1) * TCH]
                    )
                if jt == 0:
                    nc.vector.tensor_copy(es[:], ex[:])
                else:
                    nc.vector.tensor_add(es[:], es[:], ex[:])
                exs[jt] = ex

            for jt in range(min(LOOKAHEAD, njt)):
                score(jt)
            for jt in range(njt):
                if pump is not None:
                    next(pump, None)
                if jt + LOOKAHEAD < njt:
                    score(jt + LOOKAHEAD)
                nc.tensor.matmul(
                    pv[:], V_res[:, b * 8 + jt, :], exs[jt][:],
                    start=(jt == 0), stop=(jt == njt - 1),
                )
                exs[jt] = None
            psd = pools["pss"].tile([128, TCH], FP32, tag="pss")
            nc.tensor.matmul(psd[:], ones_sb[:], es[:], start=True, stop=True)
            rec = pools["rec"].tile([128, TCH], FP32, tag="rec")
            nc.vector.reciprocal_approx_fast(rec[:], psd[:])
            nc.vector.tensor_mul(ao[:, g, :], pv[:], rec[:])

        def load_wo(tci, e):
            wo_t = pools["wo"].tile(
                [128, GQ, TCH], BF16, tag="wo", name=f"wo{tci}_{e}"
            )
            nc.sync.dma_start(wo_t[:], wo16.ap()[:, :, e * TCH:(e + 1) * TCH])
            return wo_t

        def p3_gen(tci, ao):
            """generator: each next() emits one [128-token x 512-col] output
            quad of the projection for chunk tci; pumped from inside the
            attention jt loop so the PE always has filler matmuls. wo slices
            are prefetched one e-slice ahead; PSUM is drained in halves on
            ACT and DVE so neither engine queue grows a long blocker."""
            wo_t = load_wo(tci, 0)
            for e in range(8):
                esl = slice(e * TCH, (e + 1) * TCH)
                wo_next = load_wo(tci, e + 1) if e + 1 < 8 else None
                for tb in range(4):
                    pso = pools["pso"].tile([128, TCH], FP32, tag="pso", name=f"pso{tci}_{e}_{tb}")
                    for g in range(GQ):
                        nc.tensor.matmul(
                            pso[:], ao[:, g, tb * 128:(tb + 1) * 128], wo_t[:, g, :],
                            start=(g == 0), stop=(g == GQ - 1),
                        )
                    ob = pools["ob"].tile([128, TCH], BF16, tag="ob", name=f"ob{tci}_{e}_{tb}")
                    nc.scalar.copy(ob[:, 0:TCH // 2], pso[:, 0:TCH // 2])
                    nc.vector.tensor_copy(ob[:, TCH // 2:], pso[:, TCH // 2:])
                    nc.sync.dma_start(
                        outp.ap()[tci * TCH + tb * 128: tci * TCH + (tb + 1) * 128, esl],
                        ob[:],
                    )
                    yield
                wo_t = wo_next

        prev = None  # p3 generator of previous chunk
        for tci in range(NTCH):
            qT = [None] * GQ
            p1(tci, qT)
            ao = pools["ao"].tile([128, GQ, TCH], BF16, tag="ao", name=f"ao{tci}")
            for g in range(GQ):
                attn_head(tci, g, qT, ao, prev)
                if prev is not None:
                    next(prev, None)
                    next(prev, None)
            if prev is not None:
                for _ in prev:
                    pass
            prev = p3_gen(tci, ao)
        for _ in prev:
            pass

    nc.finalize()
    return nc


def _get_program(split_kv: bool):
    if split_kv not in _PROG_CACHE:
        _PROG_CACHE[split_kv] = _build_program(split_kv)
    return _PROG_CACHE[split_kv]


def kernel(
    hidden_states, wq, wk, wv, wo, kv_cache, position_ids,
    kv_page_indices, kv_page_indptr, kv_last_page_lens, qo_indptr,
    _run_kwargs: dict | None = None,
):
    hidden_states = np.asarray(hidden_states, np.float32)
    wq = np.asarray(wq, np.float32)
    wk = np.asarray(wk, np.float32)
    wv = np.asarray(wv, np.float32)
    wo = np.asarray(wo, np.float32)
    position_ids = np.asarray(position_ids, np.int32)
    qo_indptr = np.asarray(qo_indptr, np.int64)

    nnz = hidden_states.shape[0]
    b = qo_indptr.shape[0] - 1
    assert nnz == T and b == B, (nnz, b)
    assert np.array_equal(qo_indptr, np.arange(B + 1, dtype=np.int64) * S), (
        "kernel assumes uniform sequence lengths of 1024"
    )

    # Page-gather order: the reference gathers pages in list order, so the
    # token with position p within its sequence lands at page-order rank p.
    # KV must be fed in rank order; the q path stays in token order.
    perm = np.empty(T, np.int64)
    identity = True
    for bi in range(B):
        pos_b = position_ids[bi * S:(bi + 1) * S].astype(np.int64)
        assert np.array_equal(np.sort(pos_b), np.arange(S)), (
            "kernel assumes positions cover 0..S-1 exactly once per sequence"
        )
        inv = np.empty(S, np.int64)
        inv[pos_b] = np.arange(S)
        perm[bi * S:(bi + 1) * S] = bi * S + inv
        if not np.array_equal(inv, np.arange(S)):
            identity = False

    h16 = _tile_h(hidden_states)
    cosq, sinq = _rope_tables(position_ids)
    maskd = _masks_np()
    ones = np.ones((128, 128), np.float32)

    split_kv = not identity
    nc = _get_program(split_kv)

    in_maps = []
    for c in range(NCORES):
        im = {
            "h16": h16,
            "wq16": _tile_w(np.ascontiguousarray(wq[c * DQ:(c + 1) * DQ, :].T)),
            "wk16": _tile_w(np.ascontiguousarray(wk[c * HD:(c + 1) * HD, :].T)),
            "wv16": _tile_w(np.ascontiguousarray(wv[c * HD:(c + 1) * HD, :].T)),
            "wo16": _tile_wo(np.ascontiguousarray(wo[:, c * DQ:(c + 1) * DQ].T)),
            "cosq": cosq,
            "sinq": sinq,
            "maskd": maskd,
            "onesd": ones,
        }
        if split_kv:
            im["h16kv"] = _tile_h(hidden_states[perm])
            cosk, sink = _rope_tables(position_ids[perm])
            im["cosk"] = cosk
            im["sink"] = sink
        in_maps.append(im)

    res = run_bass_kernel_spmd(
        nc, in_maps, core_ids=list(range(NCORES)), **(_run_kwargs or {})
    )
    out = np.zeros((T, HID), np.float64)
    for c in range(NCORES):
        out += res.results[c]["outp"].astype(np.float64)
    kernel.last_results = res  # type: ignore[attr-defined]
    return out.astype(np.float32)
